# revision 1
# baseline (speedup 1.0000x reference)
"""CompressedAttention kernel for 8 trn2 NeuronCores.

Sharding: batch b = core//4 (data parallel), within a batch-group of 4
cores: phase A (importance) is split by q-heads (4 heads/core, 2 kv
heads/core) and combined with a 4-core AllReduce; phase B (topk select +
cumsum + interleave scatter) is replicated per core on the importance
vector, with the memory-heavy row movement split by channel chunk
(512 of 2048 channels per core).
"""
import numpy as np

import concourse.bass as bass
import concourse.mybir as mybir
import concourse.tile as tile
from concourse.vector_clock import ScopedClock
from concourse.tile_rust import add_dep_helper
from concourse.bass_utils import run_bass_kernel_spmd

F32 = mybir.dt.float32
I32 = mybir.dt.int32

# problem shape (hardcoded per contract)
B, H, KVH, D = 2, 16, 8, 128
T_W, T_CMP = 256, 4096
T_M, C = 8192, 2048
NUM_SEL = 1024
NKEEP = T_CMP - NUM_SEL          # 3072
T_OUT = T_CMP + NUM_SEL          # 5120
SCALE = D ** -0.5
N_CORES = 8
GROUP = 4                        # cores per batch
CC = C // GROUP                  # 512 channels per core
HPC = H // GROUP                 # 4 q heads per core
NT = HPC * (T_W // 128)          # 8 (head, qtile) tiles per core
SHIFTS = [24, 17, 10, 3, 0]
BIG = 1 << 20


_WS_N = [0]


def _split_waits(nc, limit=1):
    """The walrus build here rejects instructions with more than `limit`
    sync waits.  Move excess waits onto single-wait NoOps inserted just
    before the instruction on the same engine (waits gate dispatch, so
    hoisting them earlier in the same engine stream is equivalent)."""
    for bb in nc.main_func.blocks:
        new = []
        for inst in bb.instructions:
            si = getattr(inst, "sync_info", None)
            if si is not None and len(si.on_wait) > limit:
                waits = list(si.on_wait)
                si_t = type(si)
                for w in waits[:-limit]:
                    _WS_N[0] += 1
                    nop = mybir.InstNoOp(name=f"I-wsplit{_WS_N[0]}", ins=[], outs=[])
                    nop.engine = inst.engine
                    nop.sync_info = si_t(on_wait=[w], on_update=[])
                    new.append(nop)
                inst.sync_info = si_t(on_wait=waits[-limit:], on_update=list(si.on_update))
            new.append(inst)
        bb.instructions[:] = new


class TC(tile.TileContext):
    """TileContext with a walrus-compatible exit drain.

    This container's walrus build rejects InstDrain carrying multiple
    sync waits ("Too many sync wait commands"), so re-emit the exit
    clock waits as single-wait NoOps ahead of a bare drain.
    """

    def __exit__(self, et, ev, tb):
        r = super().__exit__(et, ev, tb)
        if et is None:
            _split_waits(self.nc)
        return r

    def _drain_and_barrier(self, tick_clock, wait_clock):
        waiter = self.nc.sync.nop(nofuse=True)
        wait_clock.add_sem_waits(waiter.ins, ScopedClock({None: tick_clock.global_clock}))
        si = waiter.ins.sync_info
        if si is not None and len(si.on_wait) > 1:
            waits = list(si.on_wait)
            si_t = type(si)
            waiter.ins.sync_info = si_t(on_wait=[waits[0]], on_update=list(si.on_update))
            for w in waits[1:]:
                n = self.nc.sync.nop(nofuse=True)
                n.ins.sync_info = si_t(on_wait=[w], on_update=[])
        self.nc.sync.drain()
        self.nc.all_engine_barrier()
        popped = self.nc._tile_sem_poison_stack.pop()
        assert popped is self._sem_poison
        self.nc.clear_and_free_semaphores(list(self.sems.allocated().values()))
        self.nc.all_engine_barrier()


def build():
    nc = bass.Bass(num_devices=N_CORES)
    q = nc.declare_dram_parameter("q", [HPC, T_W, D], F32, isOutput=False)
    km = nc.declare_dram_parameter("km", [HPC // 2, T_CMP, D], F32, isOutput=False)
    xm = nc.declare_dram_parameter("xm", [T_CMP, CC], F32, isOutput=False)
    xp = nc.declare_dram_parameter("xp", [T_M, CC], F32, isOutput=False)
    tri = nc.declare_dram_parameter("tri", [128, 128], F32, isOutput=False)
    iota = nc.declare_dram_parameter("iota", [128, 32], F32, isOutput=False)
    ident = nc.declare_dram_parameter("ident", [128, 128], F32, isOutput=False)
    onec = nc.declare_dram_parameter("onec", [128, 1], F32, isOutput=False)
    oner = nc.declare_dram_parameter("oner", [1, 128], F32, isOutput=False)
    iosh = nc.declare_dram_parameter("iosh", [128, len(SHIFTS)], I32, isOutput=False)
    y = nc.declare_dram_parameter("y", [T_OUT, CC], F32, isOutput=True)
    dbg = nc.declare_dram_parameter("dbg", [128, 64], F32, isOutput=True)

    ar_in = nc.dram_tensor("ar_in", [1, T_CMP], F32)
    ar_out = nc.dram_tensor("ar_out", [1, T_CMP], F32)
    comp_sel2 = nc.dram_tensor("comp_sel2", [1, NUM_SEL], I32)
    comp_selst = nc.dram_tensor("comp_selst", [1, NUM_SEL], I32)
    comp_keep = nc.dram_tensor("comp_keep", [1, NKEEP], I32)
    comp_keepst = nc.dram_tensor("comp_keepst", [1, NKEEP], I32)

    with TC(nc) as tc:
        persist_cm = tc.tile_pool(name="persist", bufs=1)
        pp = persist_cm.__enter__()
        if True:
            id_t = pp.tile([128, 128], F32)
            nc.sync.dma_start(out=id_t[:], in_=ident[:])
            expall = pp.tile([128, NT * T_CMP], F32)    # 8 x [128,4096]
            rall = pp.tile([128, NT], F32)
            zacc = pp.tile([128, NT * 8], F32)
            imp_sb = pp.tile([1, T_CMP], F32)

            # ---- phase A: scores -> exp -> Z -> r ----
            with tc.tile_pool(name="phA", bufs=2) as pa, \
                 tc.tile_pool(name="phA1", bufs=1) as pa1, \
                 tc.tile_pool(name="psA", bufs=3, space="PSUM") as psA:
                qT = pa1.tile([128, NT * 128], F32)
                kmT = pa1.tile([128, 2 * T_CMP], F32)
                # stage q (1 DMA) and km (8 DMAs of 8 tiles each) instead of
                # 72 single-tile loads — the SP sequencer cost per dma_start
                # (~1.2us) dominated phase A.
                kall = km[:].rearrange("v tp d -> (v tp) d").rearrange("(t p) d -> p t d", p=128)
                qstage = pa.tile([128, NT * 128], F32, tag="kst")
                nc.sync.dma_start(out=qstage[:].rearrange("p (t d) -> p t d", d=128), in_=q[:].rearrange("h tp d -> (h tp) d").rearrange("(t p) d -> p t d", p=128))
                for ht in range(NT):
                    tps = psA.tile([128, 128], F32, tag="tp")
                    nc.tensor.transpose(out=tps[:], in_=qstage[:, ht * 128:(ht + 1) * 128], identity=id_t[:])
                    nc.vector.tensor_copy(qT[:, ht * 128:(ht + 1) * 128], tps[:])
                for blk in range(8):
                    kstage = pa.tile([128, 8 * 128], F32, tag="kst")
                    nc.sync.dma_start(out=kstage[:].rearrange("p (t d) -> p t d", d=128), in_=kall[:, blk * 8:(blk + 1) * 8, :])
                    for j in range(8):
                        kt = blk * 8 + j
                        tps = psA.tile([128, 128], F32, tag="tp")
                        nc.tensor.transpose(out=tps[:], in_=kstage[:, j * 128:(j + 1) * 128], identity=id_t[:])
                        nc.vector.tensor_copy(kmT[:, kt * 128:(kt + 1) * 128], tps[:])
                for ht in range(NT):
                    kv = ht // 4
                    for cch in range(8):
                        scp = psA.tile([128, 512], F32, tag="sc")
                        nc.tensor.matmul(
                            out=scp[:],
                            lhsT=qT[:, ht * 128:(ht + 1) * 128],
                            rhs=kmT[:, kv * T_CMP + cch * 512: kv * T_CMP + (cch + 1) * 512],
                            start=True, stop=True)
                        nc.scalar.activation(
                            out=expall[:, ht * T_CMP + cch * 512: ht * T_CMP + (cch + 1) * 512],
                            in_=scp[:],
                            func=mybir.ActivationFunctionType.Exp,
                            scale=SCALE,
                            accum_out=zacc[:, ht * 8 + cch: ht * 8 + cch + 1])
                for ht in range(NT):
                    ztot = pa.tile([128, 1], F32, tag="ztot")
                    nc.vector.reduce_sum(ztot[:], zacc[:, ht * 8:(ht + 1) * 8], axis=mybir.AxisListType.X)
                    nc.vector.reciprocal(rall[:, ht:ht + 1], ztot[:])

            # ---- phase A2: importance = sum_ht r_ht^T @ exp_ht ----
            with tc.tile_pool(name="psA2", bufs=2, space="PSUM") as psA2:
                for cch in range(8):
                    impp = psA2.tile([1, 512], F32, tag="imp")
                    for ht in range(NT):
                        nc.tensor.matmul(
                            out=impp[:],
                            lhsT=rall[:, ht:ht + 1],
                            rhs=expall[:, ht * T_CMP + cch * 512: ht * T_CMP + (cch + 1) * 512],
                            start=(ht == 0), stop=(ht == NT - 1))
                    nc.vector.tensor_copy(imp_sb[:, cch * 512:(cch + 1) * 512], impp[:])

            dma_in = nc.sync.dma_start(out=ar_in[:], in_=imp_sb[:])
            import os as _os
            if _os.environ.get("KNOCC"):
                cc = nc.sync.dma_start(out=ar_out[:], in_=ar_in[:])
            else:
                if _os.environ.get("KSINGLE"):
                    groups = [[i] for i in range(N_CORES)]
                else:
                    groups = [[0, 1, 2, 3], [4, 5, 6, 7]]
                cc = nc.gpsimd.collective_compute(
                    "AllReduce", mybir.AluOpType.add,
                    replica_groups=groups,
                    ins=[ar_in[:]], outs=[ar_out[:]])
            add_dep_helper(cc.ins, dma_in.ins, reason="cc after ar_in write")

            persist_cm.__exit__(None, None, None)

            # ---- phase B ----
            kphase = _os.environ.get("KPHASE", "")
            with tc.tile_pool(name="phB", bufs=1) as pb, \
                 tc.tile_pool(name="psB", bufs=2, space="PSUM") as psB:
                v_nat = pb.tile([128, 32], F32)
                v_flat = pb.tile([1, T_CMP], F32)
                ld1 = nc.sync.dma_start(out=v_nat[:], in_=ar_out[:].rearrange("a (p f) -> (a p) f", p=128))
                ld2 = nc.sync.dma_start(out=v_flat[:], in_=ar_out[:])
                add_dep_helper(ld1.ins, cc.ins, reason="v_nat after allreduce")
                add_dep_helper(ld2.ins, cc.ins, reason="v_flat after allreduce")
                oner_t = pb.tile([1, 128], F32)
                onec_t = pb.tile([128, 1], F32)
                tri_t = pb.tile([128, 128], F32)
                iota_t = pb.tile([128, 32], F32)
                iosh_t = pb.tile([128, len(SHIFTS)], I32)
                nc.sync.dma_start(out=oner_t[:], in_=oner[:])
                nc.sync.dma_start(out=onec_t[:], in_=onec[:])
                nc.sync.dma_start(out=tri_t[:], in_=tri[:])
                nc.sync.dma_start(out=iota_t[:], in_=iota[:])
                nc.sync.dma_start(out=iosh_t[:], in_=iosh[:])

                if kphase == "a":
                    nc.sync.dma_start(out=dbg[:, 0:32], in_=v_nat[:])
                    return nc
                v_bc = pb.tile([128, T_CMP], F32)
                for cch in range(8):
                    vbp = psB.tile([128, 512], F32, tag="vb")
                    nc.tensor.matmul(out=vbp[:], lhsT=oner_t[:],
                                     rhs=v_flat[:, cch * 512:(cch + 1) * 512],
                                     start=True, stop=True)
                    nc.vector.tensor_copy(v_bc[:, cch * 512:(cch + 1) * 512], vbp[:])

                # radix select of the 1024-th largest value (positive f32
                # bitcast to i32 is order-isomorphic).
                t_lo = pb.tile([128, 1], I32)
                nc.vector.memset(t_lo[:], 0)
                cmp = pb.tile([128, T_CMP], F32)
                for p_i in range(len(SHIFTS)):
                    ti = pb.tile([128, 1], I32, tag=f"ti{p_i}")
                    nc.vector.tensor_tensor(out=ti[:], in0=t_lo[:], in1=iosh_t[:, p_i:p_i + 1], op=mybir.AluOpType.add)
                    nc.vector.tensor_scalar(out=cmp[:], in0=v_bc[:], scalar1=ti[:].bitcast(F32), scalar2=None, op0=mybir.AluOpType.is_ge)
                    counts = pb.tile([128, 1], F32, tag=f"cn{p_i}")
                    nc.vector.reduce_sum(counts[:], cmp[:], axis=mybir.AxisListType.X)
                    mask = pb.tile([128, 1], F32, tag=f"mk{p_i}")
                    nc.vector.tensor_scalar(out=mask[:], in0=counts[:], scalar1=float(NUM_SEL), scalar2=None, op0=mybir.AluOpType.is_ge)
                    msp = psB.tile([128, 1], F32, tag="ms")
                    nc.tensor.matmul(out=msp[:], lhsT=mask[:].to_broadcast([128, 128]), rhs=onec_t[:], start=True, stop=True)
                    js = pb.tile([128, 1], F32, tag=f"js{p_i}")
                    nc.vector.tensor_scalar(out=js[:], in0=msp[:], scalar1=-1.0, scalar2=None, op0=mybir.AluOpType.add)
                    jsi = pb.tile([128, 1], I32, tag=f"ji{p_i}")
                    nc.vector.tensor_copy(jsi[:], js[:])
                    sh = pb.tile([128, 1], I32, tag=f"sh{p_i}")
                    nc.vector.tensor_scalar(out=sh[:], in0=jsi[:], scalar1=SHIFTS[p_i], scalar2=None, op0=mybir.AluOpType.logical_shift_left)
                    nc.vector.tensor_tensor(out=t_lo[:], in0=t_lo[:], in1=sh[:], op=mybir.AluOpType.add)

                if kphase == "sel":
                    nc.sync.dma_start(out=dbg[:, 0:1], in_=t_lo[:].bitcast(F32))
                    return nc
                # membership mask on natural layout [128, 32]
                m = pb.tile([128, 32], F32)
                nc.vector.tensor_scalar(out=m[:], in0=v_nat[:], scalar1=t_lo[:].bitcast(F32), scalar2=None, op0=mybir.AluOpType.is_ge)

                # inclusive cumsum of m along linear index i = 32*p + f
                csa = pb.tile([128, 32], F32)
                csb = pb.tile([128, 32], F32)
                nc.vector.tensor_copy(csa[:], m[:])
                cur, nxt = csa, csb
                for d in [1, 2, 4, 8, 16]:
                    nc.vector.tensor_tensor(out=nxt[:, d:32], in0=cur[:, d:32], in1=cur[:, 0:32 - d], op=mybir.AluOpType.add)
                    nc.vector.tensor_copy(nxt[:, 0:d], cur[:, 0:d])
                    cur, nxt = nxt, cur
                rpp = psB.tile([128, 1], F32, tag="rp")
                nc.tensor.matmul(out=rpp[:], lhsT=tri_t[:], rhs=cur[:, 31:32], start=True, stop=True)
                rp_sb = pb.tile([128, 1], F32)
                nc.vector.tensor_copy(rp_sb[:], rpp[:])
                csg = pb.tile([128, 32], F32)
                nc.vector.tensor_scalar(out=csg[:], in0=cur[:], scalar1=rp_sb[:], scalar2=None, op0=mybir.AluOpType.add)

                rank_sel = pb.tile([128, 32], F32)
                nc.vector.tensor_tensor(out=rank_sel[:], in0=csg[:], in1=m[:], op=mybir.AluOpType.subtract)
                start_t = pb.tile([128, 32], F32)
                nc.vector.tensor_tensor(out=start_t[:], in0=rank_sel[:], in1=iota_t[:], op=mybir.AluOpType.add)
                rank_keep = pb.tile([128, 32], F32)
                nc.vector.tensor_tensor(out=rank_keep[:], in0=iota_t[:], in1=rank_sel[:], op=mybir.AluOpType.subtract)

                dropnk = pb.tile([128, 32], F32)   # BIG*(1-m)
                nc.vector.tensor_scalar(out=dropnk[:], in0=m[:], scalar1=float(-BIG), scalar2=float(BIG), op0=mybir.AluOpType.mult, op1=mybir.AluOpType.add)
                dropsel = pb.tile([128, 32], F32)  # BIG*m
                nc.vector.tensor_scalar(out=dropsel[:], in0=m[:], scalar1=float(BIG), scalar2=None, op0=mybir.AluOpType.mult)

                # pair stream: src pair index i (dropped when kept), dest = start
                pg_idx = pb.tile([128, 32], I32)
                tmpf = pb.tile([128, 32], F32)
                nc.vector.tensor_tensor(out=tmpf[:], in0=iota_t[:], in1=dropnk[:], op=mybir.AluOpType.add)
                nc.vector.tensor_copy(pg_idx[:], tmpf[:])
                ps_idx = pb.tile([128, 32], I32)
                tmpf2 = pb.tile([128, 32], F32)
                nc.vector.tensor_tensor(out=tmpf2[:], in0=start_t[:], in1=dropnk[:], op=mybir.AluOpType.add)
                nc.vector.tensor_copy(ps_idx[:], tmpf2[:])
                # kept stream: dest = start (dropped when selected)
                ks_idx = pb.tile([128, 32], I32)
                tmpf3 = pb.tile([128, 32], F32)
                nc.vector.tensor_tensor(out=tmpf3[:], in0=start_t[:], in1=dropsel[:], op=mybir.AluOpType.add)
                nc.vector.tensor_copy(ks_idx[:], tmpf3[:])

                if kphase == "cs":
                    nc.sync.dma_start(out=dbg[:, 0:32], in_=start_t[:])
                    return nc

                # debug out
                nc.sync.dma_start(out=dbg[:, 0:32], in_=m[:])
                nc.sync.dma_start(out=dbg[:, 32:64], in_=start_t[:])

                # ---- main data movement ([128,1] offsets per call) ----
                xpp = xp[:].rearrange("(n two) c -> n (two c)", two=2)  # [4096, 1024] pair rows
                bnd_in = nc.gpsimd.to_reg(T_CMP - 1)
                bnd_out = nc.gpsimd.to_reg(T_OUT - 1)
                with tc.tile_pool(name="mv", bufs=6) as mv:
                    for f in range(32):
                        # pair rows loaded sequentially on the sync engine
                        # (reads all rows; unselected ones are dropped at the
                        # scatter) — keeps GPSIMD for scatters only.
                        gP = mv.tile([128, 2 * CC], F32, tag="gP")
                        nc.sync.dma_start(out=gP[:], in_=xpp.rearrange("(p f) pc -> p f pc", f=32)[:, f, :])
                        nc.gpsimd.indirect_dma_start(
                            out=y[:], out_offset=bass.IndirectOffsetOnAxis(ap=ps_idx[:, f:f + 1], axis=0),
                            in_=gP[:], in_offset=None,
                            bounds_check=bnd_out, oob_is_err=False)
                        gK = mv.tile([128, CC], F32, tag="gK")
                        nc.sync.dma_start(out=gK[:], in_=xm[:].rearrange("(p f) c -> p f c", f=32)[:, f, :])
                        nc.gpsimd.indirect_dma_start(
                            out=y[:], out_offset=bass.IndirectOffsetOnAxis(ap=ks_idx[:, f:f + 1], axis=0),
                            in_=gK[:], in_offset=None,
                            bounds_check=bnd_out, oob_is_err=False)
    return nc


_NC_CACHE = None
LAST_RESULT = None


def _consts():
    tri = np.triu(np.ones((128, 128), np.float32), k=1)
    iota = (np.arange(4096, dtype=np.float32)).reshape(128, 32)
    ident = np.eye(128, dtype=np.float32)
    onec = np.ones((128, 1), np.float32)
    oner = np.ones((1, 128), np.float32)
    iosh = np.zeros((128, len(SHIFTS)), np.int32)
    for i, s in enumerate(SHIFTS):
        iosh[:, i] = np.arange(128, dtype=np.int64) << s
    return dict(tri=tri, iota=iota, ident=ident, onec=onec, oner=oner, iosh=iosh)


def kernel(q_w, km_cmp, x_m, xm_cmp):
    global _NC_CACHE
    q_w = np.ascontiguousarray(q_w, dtype=np.float32)
    km_cmp = np.ascontiguousarray(km_cmp, dtype=np.float32)
    x_m = np.ascontiguousarray(x_m, dtype=np.float32)
    xm_cmp = np.ascontiguousarray(xm_cmp, dtype=np.float32)
    if _NC_CACHE is None:
        _NC_CACHE = build()
    nc = _NC_CACHE
    cst = _consts()
    in_maps = []
    for c in range(N_CORES):
        b, j = c // GROUP, c % GROUP
        c0 = j * CC
        in_maps.append({
            "q": np.ascontiguousarray(q_w[b, j * HPC:(j + 1) * HPC]),
            "km": np.ascontiguousarray(km_cmp[b, j * HPC // 2:(j + 1) * HPC // 2]),
            "xm": np.ascontiguousarray(xm_cmp[b, :, c0:c0 + CC]),
            "xp": np.ascontiguousarray(x_m[b, :, c0:c0 + CC]),
            **cst,
        })
    res = run_bass_kernel_spmd(nc, in_maps, core_ids=list(range(N_CORES)))
    global LAST_RESULT
    LAST_RESULT = res
    out = np.empty((B, T_OUT, C), np.float32)
    for c in range(N_CORES):
        b, j = c // GROUP, c % GROUP
        out[b, :, j * CC:(j + 1) * CC] = res.results[c]["y"]
    return out



# revision 2
# speedup vs baseline: 4.9816x; 4.9816x over previous
"""CompressedAttention kernel for 8 trn2 NeuronCores.

Sharding: batch b = core//4 (data parallel), within a batch-group of 4
cores: phase A (importance) is split by q-heads (4 heads/core, 2 kv
heads/core) and combined with a 4-core AllReduce; phase B (topk select +
cumsum + interleave scatter) is replicated per core on the importance
vector, with the memory-heavy row movement split by channel chunk
(512 of 2048 channels per core).
"""
import numpy as np

import concourse.bass as bass
import concourse.mybir as mybir
import concourse.tile as tile
from concourse.vector_clock import ScopedClock
from concourse.tile_rust import add_dep_helper
from concourse.bass_utils import run_bass_kernel_spmd

F32 = mybir.dt.float32
I32 = mybir.dt.int32

# problem shape (hardcoded per contract)
B, H, KVH, D = 2, 16, 8, 128
T_W, T_CMP = 256, 4096
T_M, C = 8192, 2048
NUM_SEL = 1024
NKEEP = T_CMP - NUM_SEL          # 3072
T_OUT = T_CMP + NUM_SEL          # 5120
SCALE = D ** -0.5
N_CORES = 8
GROUP = 4                        # cores per batch
CC = C // GROUP                  # 512 channels per core
HPC = H // GROUP                 # 4 q heads per core
NT = HPC * (T_W // 128)          # 8 (head, qtile) tiles per core
SHIFTS = [24, 17, 10, 3, 0]
BIG = 1 << 20


_WS_N = [0]


def _split_waits(nc, limit=1):
    """The walrus build here rejects instructions with more than `limit`
    sync waits.  Move excess waits onto single-wait NoOps inserted just
    before the instruction on the same engine (waits gate dispatch, so
    hoisting them earlier in the same engine stream is equivalent)."""
    for bb in nc.main_func.blocks:
        new = []
        for inst in bb.instructions:
            si = getattr(inst, "sync_info", None)
            if si is not None and len(si.on_wait) > limit:
                waits = list(si.on_wait)
                si_t = type(si)
                for w in waits[:-limit]:
                    _WS_N[0] += 1
                    nop = mybir.InstNoOp(name=f"I-wsplit{_WS_N[0]}", ins=[], outs=[])
                    nop.engine = inst.engine
                    nop.sync_info = si_t(on_wait=[w], on_update=[])
                    new.append(nop)
                inst.sync_info = si_t(on_wait=waits[-limit:], on_update=list(si.on_update))
            new.append(inst)
        bb.instructions[:] = new


class TC(tile.TileContext):
    """TileContext with a walrus-compatible exit drain.

    This container's walrus build rejects InstDrain carrying multiple
    sync waits ("Too many sync wait commands"), so re-emit the exit
    clock waits as single-wait NoOps ahead of a bare drain.
    """

    def __exit__(self, et, ev, tb):
        r = super().__exit__(et, ev, tb)
        if et is None:
            _split_waits(self.nc)
        return r

    def _drain_and_barrier(self, tick_clock, wait_clock):
        waiter = self.nc.sync.nop(nofuse=True)
        wait_clock.add_sem_waits(waiter.ins, ScopedClock({None: tick_clock.global_clock}))
        si = waiter.ins.sync_info
        if si is not None and len(si.on_wait) > 1:
            waits = list(si.on_wait)
            si_t = type(si)
            waiter.ins.sync_info = si_t(on_wait=[waits[0]], on_update=list(si.on_update))
            for w in waits[1:]:
                n = self.nc.sync.nop(nofuse=True)
                n.ins.sync_info = si_t(on_wait=[w], on_update=[])
        self.nc.sync.drain()
        self.nc.all_engine_barrier()
        popped = self.nc._tile_sem_poison_stack.pop()
        assert popped is self._sem_poison
        self.nc.clear_and_free_semaphores(list(self.sems.allocated().values()))
        self.nc.all_engine_barrier()


def build():
    nc = bass.Bass(num_devices=N_CORES)
    q = nc.declare_dram_parameter("q", [HPC, T_W, D], F32, isOutput=False)
    km = nc.declare_dram_parameter("km", [HPC // 2, T_CMP, D], F32, isOutput=False)
    xm = nc.declare_dram_parameter("xm", [T_CMP, CC], F32, isOutput=False)
    xp = nc.declare_dram_parameter("xp", [T_M, CC], F32, isOutput=False)
    tri = nc.declare_dram_parameter("tri", [128, 128], F32, isOutput=False)
    iota = nc.declare_dram_parameter("iota", [128, 32], F32, isOutput=False)
    ident = nc.declare_dram_parameter("ident", [128, 128], F32, isOutput=False)
    onec = nc.declare_dram_parameter("onec", [128, 1], F32, isOutput=False)
    oner = nc.declare_dram_parameter("oner", [1, 128], F32, isOutput=False)
    iosh = nc.declare_dram_parameter("iosh", [128, len(SHIFTS)], I32, isOutput=False)
    y = nc.declare_dram_parameter("y", [T_OUT, CC], F32, isOutput=True)
    dbg = nc.declare_dram_parameter("dbg", [128, 64], F32, isOutput=True)

    ar_in = nc.dram_tensor("ar_in", [1, T_CMP], F32)
    ar_out = nc.dram_tensor("ar_out", [1, T_CMP], F32)
    comp_sel2 = nc.dram_tensor("comp_sel2", [1, NUM_SEL], I32)
    comp_selst = nc.dram_tensor("comp_selst", [1, NUM_SEL], I32)
    comp_keep = nc.dram_tensor("comp_keep", [1, NKEEP], I32)
    comp_keepst = nc.dram_tensor("comp_keepst", [1, NKEEP], I32)

    with TC(nc) as tc:
        persist_cm = tc.tile_pool(name="persist", bufs=1)
        pp = persist_cm.__enter__()
        if True:
            id_t = pp.tile([128, 128], F32)
            nc.sync.dma_start(out=id_t[:], in_=ident[:])
            expall = pp.tile([128, NT * T_CMP], F32)    # 8 x [128,4096]
            rall = pp.tile([128, NT], F32)
            zacc = pp.tile([128, NT * 8], F32)
            imp_sb = pp.tile([1, T_CMP], F32)

            # ---- phase A: scores -> exp -> Z -> r ----
            with tc.tile_pool(name="phA", bufs=2) as pa, \
                 tc.tile_pool(name="phA1", bufs=1) as pa1, \
                 tc.tile_pool(name="psA", bufs=3, space="PSUM") as psA:
                qT = pa1.tile([128, NT * 128], F32)
                kmT = pa1.tile([128, 2 * T_CMP], F32)
                # stage q (1 DMA) and km (8 DMAs of 8 tiles each) instead of
                # 72 single-tile loads — the SP sequencer cost per dma_start
                # (~1.2us) dominated phase A.
                kall = km[:].rearrange("v tp d -> (v tp) d").rearrange("(t p) d -> p t d", p=128)
                qstage = pa.tile([128, NT * 128], F32, tag="kst")
                nc.sync.dma_start(out=qstage[:].rearrange("p (t d) -> p t d", d=128), in_=q[:].rearrange("h tp d -> (h tp) d").rearrange("(t p) d -> p t d", p=128))
                for ht in range(NT):
                    tps = psA.tile([128, 128], F32, tag="tp")
                    nc.tensor.transpose(out=tps[:], in_=qstage[:, ht * 128:(ht + 1) * 128], identity=id_t[:])
                    nc.vector.tensor_copy(qT[:, ht * 128:(ht + 1) * 128], tps[:])
                for blk in range(8):
                    kstage = pa.tile([128, 8 * 128], F32, tag="kst")
                    nc.sync.dma_start(out=kstage[:].rearrange("p (t d) -> p t d", d=128), in_=kall[:, blk * 8:(blk + 1) * 8, :])
                    for j in range(8):
                        kt = blk * 8 + j
                        tps = psA.tile([128, 128], F32, tag="tp")
                        nc.tensor.transpose(out=tps[:], in_=kstage[:, j * 128:(j + 1) * 128], identity=id_t[:])
                        nc.vector.tensor_copy(kmT[:, kt * 128:(kt + 1) * 128], tps[:])
                for ht in range(NT):
                    kv = ht // 4
                    for cch in range(8):
                        scp = psA.tile([128, 512], F32, tag="sc")
                        nc.tensor.matmul(
                            out=scp[:],
                            lhsT=qT[:, ht * 128:(ht + 1) * 128],
                            rhs=kmT[:, kv * T_CMP + cch * 512: kv * T_CMP + (cch + 1) * 512],
                            start=True, stop=True)
                        nc.scalar.activation(
                            out=expall[:, ht * T_CMP + cch * 512: ht * T_CMP + (cch + 1) * 512],
                            in_=scp[:],
                            func=mybir.ActivationFunctionType.Exp,
                            scale=SCALE,
                            accum_out=zacc[:, ht * 8 + cch: ht * 8 + cch + 1])
                for ht in range(NT):
                    ztot = pa.tile([128, 1], F32, tag="ztot")
                    nc.vector.reduce_sum(ztot[:], zacc[:, ht * 8:(ht + 1) * 8], axis=mybir.AxisListType.X)
                    nc.vector.reciprocal(rall[:, ht:ht + 1], ztot[:])

            # ---- phase A2: importance = sum_ht r_ht^T @ exp_ht ----
            with tc.tile_pool(name="psA2", bufs=2, space="PSUM") as psA2:
                for cch in range(8):
                    impp = psA2.tile([1, 512], F32, tag="imp")
                    for ht in range(NT):
                        nc.tensor.matmul(
                            out=impp[:],
                            lhsT=rall[:, ht:ht + 1],
                            rhs=expall[:, ht * T_CMP + cch * 512: ht * T_CMP + (cch + 1) * 512],
                            start=(ht == 0), stop=(ht == NT - 1))
                    nc.vector.tensor_copy(imp_sb[:, cch * 512:(cch + 1) * 512], impp[:])

            dma_in = nc.sync.dma_start(out=ar_in[:], in_=imp_sb[:])
            import os as _os
            if _os.environ.get("KNOCC"):
                cc = nc.sync.dma_start(out=ar_out[:], in_=ar_in[:])
            else:
                if _os.environ.get("KSINGLE"):
                    groups = [[i] for i in range(N_CORES)]
                else:
                    groups = [[0, 1, 2, 3], [4, 5, 6, 7]]
                cc = nc.gpsimd.collective_compute(
                    "AllReduce", mybir.AluOpType.add,
                    replica_groups=groups,
                    ins=[ar_in[:]], outs=[ar_out[:]])
            add_dep_helper(cc.ins, dma_in.ins, reason="cc after ar_in write")

            persist_cm.__exit__(None, None, None)

            # ---- phase B ----
            kphase = _os.environ.get("KPHASE", "")
            with tc.tile_pool(name="phB", bufs=1) as pb, \
                 tc.tile_pool(name="psB", bufs=2, space="PSUM") as psB:
                v_nat = pb.tile([128, 32], F32)
                v_flat = pb.tile([1, T_CMP], F32)
                ld1 = nc.sync.dma_start(out=v_nat[:], in_=ar_out[:].rearrange("a (p f) -> (a p) f", p=128))
                ld2 = nc.sync.dma_start(out=v_flat[:], in_=ar_out[:])
                add_dep_helper(ld1.ins, cc.ins, reason="v_nat after allreduce")
                add_dep_helper(ld2.ins, cc.ins, reason="v_flat after allreduce")
                oner_t = pb.tile([1, 128], F32)
                onec_t = pb.tile([128, 1], F32)
                tri_t = pb.tile([128, 128], F32)
                iota_t = pb.tile([128, 32], F32)
                iosh_t = pb.tile([128, len(SHIFTS)], I32)
                nc.sync.dma_start(out=oner_t[:], in_=oner[:])
                nc.sync.dma_start(out=onec_t[:], in_=onec[:])
                nc.sync.dma_start(out=tri_t[:], in_=tri[:])
                nc.sync.dma_start(out=iota_t[:], in_=iota[:])
                nc.sync.dma_start(out=iosh_t[:], in_=iosh[:])

                if kphase == "a":
                    nc.sync.dma_start(out=dbg[:, 0:32], in_=v_nat[:])
                    return nc
                v_bc = pb.tile([128, T_CMP], F32)
                for cch in range(8):
                    vbp = psB.tile([128, 512], F32, tag="vb")
                    nc.tensor.matmul(out=vbp[:], lhsT=oner_t[:],
                                     rhs=v_flat[:, cch * 512:(cch + 1) * 512],
                                     start=True, stop=True)
                    nc.vector.tensor_copy(v_bc[:, cch * 512:(cch + 1) * 512], vbp[:])

                # radix select of the 1024-th largest value (positive f32
                # bitcast to i32 is order-isomorphic).
                t_lo = pb.tile([128, 1], I32)
                nc.vector.memset(t_lo[:], 0)
                cmp = pb.tile([128, T_CMP], F32)
                for p_i in range(len(SHIFTS)):
                    ti = pb.tile([128, 1], I32, tag=f"ti{p_i}")
                    nc.vector.tensor_tensor(out=ti[:], in0=t_lo[:], in1=iosh_t[:, p_i:p_i + 1], op=mybir.AluOpType.add)
                    nc.vector.tensor_scalar(out=cmp[:], in0=v_bc[:], scalar1=ti[:].bitcast(F32), scalar2=None, op0=mybir.AluOpType.is_ge)
                    counts = pb.tile([128, 1], F32, tag=f"cn{p_i}")
                    nc.vector.reduce_sum(counts[:], cmp[:], axis=mybir.AxisListType.X)
                    mask = pb.tile([128, 1], F32, tag=f"mk{p_i}")
                    nc.vector.tensor_scalar(out=mask[:], in0=counts[:], scalar1=float(NUM_SEL), scalar2=None, op0=mybir.AluOpType.is_ge)
                    msp = psB.tile([128, 1], F32, tag="ms")
                    nc.tensor.matmul(out=msp[:], lhsT=mask[:].to_broadcast([128, 128]), rhs=onec_t[:], start=True, stop=True)
                    js = pb.tile([128, 1], F32, tag=f"js{p_i}")
                    nc.vector.tensor_scalar(out=js[:], in0=msp[:], scalar1=-1.0, scalar2=None, op0=mybir.AluOpType.add)
                    jsi = pb.tile([128, 1], I32, tag=f"ji{p_i}")
                    nc.vector.tensor_copy(jsi[:], js[:])
                    sh = pb.tile([128, 1], I32, tag=f"sh{p_i}")
                    nc.vector.tensor_scalar(out=sh[:], in0=jsi[:], scalar1=SHIFTS[p_i], scalar2=None, op0=mybir.AluOpType.logical_shift_left)
                    nc.vector.tensor_tensor(out=t_lo[:], in0=t_lo[:], in1=sh[:], op=mybir.AluOpType.add)

                if kphase == "sel":
                    nc.sync.dma_start(out=dbg[:, 0:1], in_=t_lo[:].bitcast(F32))
                    return nc
                # membership mask on natural layout [128, 32]
                m = pb.tile([128, 32], F32)
                nc.vector.tensor_scalar(out=m[:], in0=v_nat[:], scalar1=t_lo[:].bitcast(F32), scalar2=None, op0=mybir.AluOpType.is_ge)

                # inclusive cumsum of m along linear index i = 32*p + f
                csa = pb.tile([128, 32], F32)
                csb = pb.tile([128, 32], F32)
                nc.vector.tensor_copy(csa[:], m[:])
                cur, nxt = csa, csb
                for d in [1, 2, 4, 8, 16]:
                    nc.vector.tensor_tensor(out=nxt[:, d:32], in0=cur[:, d:32], in1=cur[:, 0:32 - d], op=mybir.AluOpType.add)
                    nc.vector.tensor_copy(nxt[:, 0:d], cur[:, 0:d])
                    cur, nxt = nxt, cur
                rpp = psB.tile([128, 1], F32, tag="rp")
                nc.tensor.matmul(out=rpp[:], lhsT=tri_t[:], rhs=cur[:, 31:32], start=True, stop=True)
                rp_sb = pb.tile([128, 1], F32)
                nc.vector.tensor_copy(rp_sb[:], rpp[:])
                csg = pb.tile([128, 32], F32)
                nc.vector.tensor_scalar(out=csg[:], in0=cur[:], scalar1=rp_sb[:], scalar2=None, op0=mybir.AluOpType.add)

                rank_sel = pb.tile([128, 32], F32)
                nc.vector.tensor_tensor(out=rank_sel[:], in0=csg[:], in1=m[:], op=mybir.AluOpType.subtract)
                start_t = pb.tile([128, 32], F32)
                nc.vector.tensor_tensor(out=start_t[:], in0=rank_sel[:], in1=iota_t[:], op=mybir.AluOpType.add)
                rank_keep = pb.tile([128, 32], F32)
                nc.vector.tensor_tensor(out=rank_keep[:], in0=iota_t[:], in1=rank_sel[:], op=mybir.AluOpType.subtract)

                dropnk = pb.tile([128, 32], F32)   # BIG*(1-m)
                nc.vector.tensor_scalar(out=dropnk[:], in0=m[:], scalar1=float(-BIG), scalar2=float(BIG), op0=mybir.AluOpType.mult, op1=mybir.AluOpType.add)
                dropsel = pb.tile([128, 32], F32)  # BIG*m
                nc.vector.tensor_scalar(out=dropsel[:], in0=m[:], scalar1=float(BIG), scalar2=None, op0=mybir.AluOpType.mult)

                # pair stream: src pair index i (dropped when kept), dest = start
                pg_idx = pb.tile([128, 32], I32)
                tmpf = pb.tile([128, 32], F32)
                nc.vector.tensor_tensor(out=tmpf[:], in0=iota_t[:], in1=dropnk[:], op=mybir.AluOpType.add)
                nc.vector.tensor_copy(pg_idx[:], tmpf[:])
                ps_idx = pb.tile([128, 32], I32)
                tmpf2 = pb.tile([128, 32], F32)
                nc.vector.tensor_tensor(out=tmpf2[:], in0=start_t[:], in1=dropnk[:], op=mybir.AluOpType.add)
                nc.vector.tensor_copy(ps_idx[:], tmpf2[:])
                # kept stream: dest = start (dropped when selected)
                ks_idx = pb.tile([128, 32], I32)
                tmpf3 = pb.tile([128, 32], F32)
                nc.vector.tensor_tensor(out=tmpf3[:], in0=start_t[:], in1=dropsel[:], op=mybir.AluOpType.add)
                nc.vector.tensor_copy(ks_idx[:], tmpf3[:])

                if kphase == "cs":
                    nc.sync.dma_start(out=dbg[:, 0:32], in_=start_t[:])
                    return nc

                # debug out
                nc.sync.dma_start(out=dbg[:, 0:32], in_=m[:])
                nc.sync.dma_start(out=dbg[:, 32:64], in_=start_t[:])

                # ---- main data movement ([128,1] offsets per call) ----
                xpp = xp[:].rearrange("(n two) c -> n (two c)", two=2)  # [4096, 1024] pair rows
                bnd_in = nc.gpsimd.to_reg(T_CMP - 1)
                bnd_out = nc.gpsimd.to_reg(T_OUT - 1)
                with tc.tile_pool(name="mv", bufs=6) as mv:
                    for f in range(32):
                        # pair rows loaded sequentially on the sync engine
                        # (reads all rows; unselected ones are dropped at the
                        # scatter) — keeps GPSIMD for scatters only.
                        gP = mv.tile([128, 2 * CC], F32, tag="gP")
                        nc.sync.dma_start(out=gP[:], in_=xpp.rearrange("(p f) pc -> p f pc", f=32)[:, f, :])
                        nc.gpsimd.indirect_dma_start(
                            out=y[0:128], out_offset=bass.IndirectOffsetOnAxis(ap=ps_idx[:, f:f + 1], axis=0),
                            in_=gP[:], in_offset=None,
                            bounds_check=bnd_out, oob_is_err=False)
                        gK = mv.tile([128, CC], F32, tag="gK")
                        nc.sync.dma_start(out=gK[:], in_=xm[:].rearrange("(p f) c -> p f c", f=32)[:, f, :])
                        nc.gpsimd.indirect_dma_start(
                            out=y[0:128], out_offset=bass.IndirectOffsetOnAxis(ap=ks_idx[:, f:f + 1], axis=0),
                            in_=gK[:], in_offset=None,
                            bounds_check=bnd_out, oob_is_err=False)
    return nc


_NC_CACHE = None
LAST_RESULT = None


def _consts():
    tri = np.triu(np.ones((128, 128), np.float32), k=1)
    iota = (np.arange(4096, dtype=np.float32)).reshape(128, 32)
    ident = np.eye(128, dtype=np.float32)
    onec = np.ones((128, 1), np.float32)
    oner = np.ones((1, 128), np.float32)
    iosh = np.zeros((128, len(SHIFTS)), np.int32)
    for i, s in enumerate(SHIFTS):
        iosh[:, i] = np.arange(128, dtype=np.int64) << s
    return dict(tri=tri, iota=iota, ident=ident, onec=onec, oner=oner, iosh=iosh)


def kernel(q_w, km_cmp, x_m, xm_cmp):
    global _NC_CACHE
    q_w = np.ascontiguousarray(q_w, dtype=np.float32)
    km_cmp = np.ascontiguousarray(km_cmp, dtype=np.float32)
    x_m = np.ascontiguousarray(x_m, dtype=np.float32)
    xm_cmp = np.ascontiguousarray(xm_cmp, dtype=np.float32)
    if _NC_CACHE is None:
        _NC_CACHE = build()
    nc = _NC_CACHE
    cst = _consts()
    in_maps = []
    for c in range(N_CORES):
        b, j = c // GROUP, c % GROUP
        c0 = j * CC
        in_maps.append({
            "q": np.ascontiguousarray(q_w[b, j * HPC:(j + 1) * HPC]),
            "km": np.ascontiguousarray(km_cmp[b, j * HPC // 2:(j + 1) * HPC // 2]),
            "xm": np.ascontiguousarray(xm_cmp[b, :, c0:c0 + CC]),
            "xp": np.ascontiguousarray(x_m[b, :, c0:c0 + CC]),
            **cst,
        })
    res = run_bass_kernel_spmd(nc, in_maps, core_ids=list(range(N_CORES)))
    global LAST_RESULT
    LAST_RESULT = res
    out = np.empty((B, T_OUT, C), np.float32)
    for c in range(N_CORES):
        b, j = c // GROUP, c % GROUP
        out[b, :, j * CC:(j + 1) * CC] = res.results[c]["y"]
    return out

